# revision 4
# baseline (speedup 1.0000x reference)
"""Trainium2 Bass kernel for nn_MultiHeadAttention_515396076443 (sparse_attention).

Math shortcut that makes this fast: the reference applies straight-through
argmax hardening, `attn = hard - stop_gradient(attn) + attn`, right before
using `attn`.  In fp32 forward arithmetic `(0 - a) + a == 0` exactly and
`(1 - a) + a == 1` to within 1 ulp, so the effective attention matrix is a
one-hot selection of the top unmasked key per query:

    j*[b,h,q] = argmax_k ( qk_scores + qk_mask + k_mask  restricted to
                           post_softmax_mask == 1 )

(the top-k/softmax/renorm steps only rescale probabilities monotonically and
cannot change the argmax; non-top-k entries get exactly 0 probability, and
row selection commutes with the fc projection).  Then

    output[b,q] = sum_h (v[b] @ w_vs_h @ fc_h)[j*[b,h,q]] + v[b,q]
    attn[b,h,q] = one_hot(j*)      (argmax entry is 1 to within 1 ulp)

Sharding: 8 cores = 4 batches x 2 head-halves (6 heads each).
QK scores use native fp32 matmuls (argmax fidelity); the value path uses
float32r (~1.4e-4 rel err, 4x faster).
"""

import numpy as np

B, LQ, LK, H, DK, DV, E = 4, 1024, 1024, 12, 64, 768, 768
HPC = 6            # heads per core
QT = LQ // 128     # 8 q-tiles
ET = E // 128      # 6
DVT = DV // 128    # 6
MT = HPC * DK // 128  # 3 head-pair tiles
N_CORES = 8

_compiled = {}


def _build_program():
    from contextlib import ExitStack
    import concourse.tile as tile
    import concourse.mybir as mybir
    from concourse import bacc

    f32 = mybir.dt.float32
    f32r = mybir.dt.float32r
    u16 = mybir.dt.uint16
    i16 = mybir.dt.int16

    nc = bacc.Bacc(None, target_bir_lowering=False)

    qT_ext = nc.dram_tensor("qT", [E, LQ], f32, kind="ExternalInput")
    kT_ext = nc.dram_tensor("kT", [E, LK], f32, kind="ExternalInput")
    vT_ext = nc.dram_tensor("vT", [DV, LK], f32, kind="ExternalInput")
    wq_ext = nc.dram_tensor("wq", [E, HPC * DK], f32, kind="ExternalInput")
    wk_ext = nc.dram_tensor("wk", [E, HPC * DK], f32, kind="ExternalInput")
    wv_ext = nc.dram_tensor("wv", [DV, HPC * DV], f32, kind="ExternalInput")
    fc_ext = nc.dram_tensor("fc", [HPC * DV, DV], f32, kind="ExternalInput")
    qk_ext = nc.dram_tensor("qk", [LQ, LK], f32, kind="ExternalInput")
    post_ext = nc.dram_tensor("post", [LQ, LK], f32, kind="ExternalInput")
    km_ext = nc.dram_tensor("km", [1, LK], f32, kind="ExternalInput")

    jout_ext = nc.dram_tensor("jout", [HPC, 128, QT], u16, kind="ExternalOutput")
    part_ext = nc.dram_tensor("part", [128, QT, DV], f32, kind="ExternalOutput")

    with tile.TileContext(nc) as tc:
        with ExitStack() as ctx:
            const = ctx.enter_context(tc.tile_pool(name="const", bufs=1))
            dram = ctx.enter_context(tc.tile_pool(name="dram", bufs=2, space="DRAM"))
            jdram = ctx.enter_context(tc.tile_pool(name="jdram", bufs=1, space="DRAM"))

            jtiles = const.tile([128, HPC, QT], u16, tag="jtiles")
            idx_all = const.tile([128, HPC * (LQ // 16)], i16, tag="idx_all")

            with ExitStack() as ab:   # pools that live through phases A+B only
                abp = ab.enter_context(tc.tile_pool(name="abp", bufs=1))

                # ---------- constants ----------
                ones_t = abp.tile([1, 128], f32, tag="ones")
                nc.vector.memset(ones_t[:], 1.0)
                km_t = abp.tile([1, LK], f32, tag="km")
                nc.gpsimd.dma_start(km_t[:], km_ext[:])

                kb_t = abp.tile([128, LK], f32, tag="kb")
                with tc.tile_pool(name="kbp", bufs=1, space="PSUM") as kbp:
                    kb_ps = kbp.tile([128, LK], f32, tag="kb_ps")
                    for c in range(2):
                        nc.tensor.matmul(kb_ps[:, c * 512:(c + 1) * 512], ones_t[:],
                                         km_t[:, c * 512:(c + 1) * 512],
                                         start=True, stop=True)
                    nc.scalar.copy(kb_t[:], kb_ps[:])

                # ---------- phase A: qhT/khT projections (fp32) ----------
                qhT_t = abp.tile([128, MT, LQ], f32, tag="qhT")
                khT_t = abp.tile([128, MT, LK], f32, tag="khT")
                with tc.tile_pool(name="pa_sb", bufs=1) as pa_sb, \
                     tc.tile_pool(name="pa_ps", bufs=2, space="PSUM") as pa_ps:
                    wq_t = pa_sb.tile([128, ET, HPC * DK], f32, tag="wq")
                    wk_t = pa_sb.tile([128, ET, HPC * DK], f32, tag="wk")
                    qTs = pa_sb.tile([128, ET, LQ], f32, tag="qTs")
                    kTs = pa_sb.tile([128, ET, LK], f32, tag="kTs")
                    for kt in range(ET):
                        nc.gpsimd.dma_start(wq_t[:, kt, :],
                                            wq_ext[kt * 128:(kt + 1) * 128, :])
                        nc.gpsimd.dma_start(wk_t[:, kt, :],
                                            wk_ext[kt * 128:(kt + 1) * 128, :])
                        nc.gpsimd.dma_start(qTs[:, kt, :],
                                            qT_ext[kt * 128:(kt + 1) * 128, :])
                        nc.gpsimd.dma_start(kTs[:, kt, :],
                                            kT_ext[kt * 128:(kt + 1) * 128, :])
                    for (w_t, src, dst) in ((wq_t, qTs, qhT_t), (wk_t, kTs, khT_t)):
                        for mt in range(MT):
                            for c in range(2):
                                ps = pa_ps.tile([128, 512], f32, tag="pa")
                                for kt in range(ET):
                                    nc.tensor.matmul(
                                        ps[:],
                                        w_t[:, kt, mt * 128:(mt + 1) * 128],
                                        src[:, kt, c * 512:(c + 1) * 512],
                                        start=(kt == 0), stop=(kt == ET - 1))
                                nc.scalar.copy(dst[:, mt, c * 512:(c + 1) * 512],
                                               ps[:])

                # ---------- phase B: scores + argmax (fp32) ----------
                with tc.tile_pool(name="pb_sb", bufs=2) as pb_sb, \
                     tc.tile_pool(name="pb_s", bufs=3) as pb_s, \
                     tc.tile_pool(name="pb_ps", bufs=4, space="PSUM") as pb_ps:
                    for qt in range(QT):
                        qk_t = pb_sb.tile([128, LK], f32, tag="qk")
                        post_t = pb_sb.tile([128, LK], f32, tag="post")
                        nc.gpsimd.dma_start(qk_t[:], qk_ext[qt * 128:(qt + 1) * 128, :])
                        nc.gpsimd.dma_start(post_t[:],
                                            post_ext[qt * 128:(qt + 1) * 128, :])
                        pen_t = pb_sb.tile([128, LK], f32, tag="pen")
                        nc.scalar.activation(pen_t[:], post_t[:],
                                             mybir.ActivationFunctionType.Copy,
                                             bias=-1e9, scale=1e9)
                        comb_t = pb_sb.tile([128, LK], f32, tag="comb")
                        nc.vector.tensor_add(comb_t[:], qk_t[:], pen_t[:])
                        nc.vector.tensor_add(comb_t[:], comb_t[:], kb_t[:])

                        for hp in range(MT):  # head pairs, packed on row groups
                            accs = [pb_ps.tile([128, LK], f32, tag="acc",
                                               name=f"acc{i}")
                                    for i in range(2)]
                            for c in range(2):
                                for hi in range(2):
                                    nc.tensor.matmul(
                                        accs[hi][:, c * 512:(c + 1) * 512],
                                        qhT_t[64 * hi:64 * (hi + 1), hp,
                                              qt * 128:(qt + 1) * 128],
                                        khT_t[64 * hi:64 * (hi + 1), hp,
                                              c * 512:(c + 1) * 512],
                                        start=True, stop=True)
                            for hi in range(2):
                                h = 2 * hp + hi
                                s_t = pb_s.tile([128, LK], f32, tag="s")
                                nc.vector.tensor_add(s_t[:], accs[hi][:], comb_t[:])
                                max8 = pb_s.tile([128, 8], f32, tag="max8")
                                idx8 = pb_s.tile([128, 8], u16, tag="idx8")
                                nc.vector.max(max8[:], s_t[:])
                                nc.vector.max_index(idx8[:], max8[:], s_t[:])
                                nc.vector.tensor_copy(jtiles[:, h, qt:qt + 1],
                                                      idx8[:, 0:1])

            # j* to DRAM in q-order flat layout, then read back in the gather
            # ucode's index layout (idx i at partition i%16, col i//16),
            # replicated into all 8 partition groups.
            jscr = jdram.tile([HPC, LQ], u16, tag="jscr")
            for h in range(HPC):
                nc.gpsimd.dma_start(jout_ext[h], jtiles[:, h, :])
                nc.gpsimd.dma_start(
                    jscr[h].rearrange("(c p) -> p c", p=128),
                    jtiles[:, h, :])
            for g in range(8):
                nc.gpsimd.dma_start(
                    idx_all[16 * g:16 * (g + 1), :],
                    jscr[:].rearrange("h (c p) -> p (h c)", p=16))

            # ---------- phase C: value path (float32r) + gather ----------
            vT_r = const.tile([128, DVT, LK], f32r, tag="vT_r")
            for kt in range(DVT):
                nc.gpsimd.dma_start(vT_r[:, kt, :], vT_ext[kt * 128:(kt + 1) * 128, :])

            part_t = const.tile([128, QT * DV], f32, tag="part")
            with tc.tile_pool(name="pc_w", bufs=2) as pc_w, \
                 tc.tile_pool(name="pc_vh", bufs=1) as pc_vh, \
                 tc.tile_pool(name="pc_ev", bufs=2) as pc_ev, \
                 tc.tile_pool(name="pc_ps", bufs=2, space="PSUM") as pc_ps, \
                 tc.tile_pool(name="pc_g", bufs=1) as pc_g:
                for h in range(HPC):
                    wv_t = pc_w.tile([128, DVT, DV], f32r, tag="wv")
                    fc_t = pc_w.tile([128, DVT, DV], f32r, tag="fc")
                    for kt in range(DVT):
                        nc.gpsimd.dma_start(
                            wv_t[:, kt, :],
                            wv_ext[kt * 128:(kt + 1) * 128, h * DV:(h + 1) * DV])
                        nc.gpsimd.dma_start(
                            fc_t[:, kt, :],
                            fc_ext[h * DV + kt * 128:h * DV + (kt + 1) * 128, :])

                    # vh^T = wv_h.T @ vT  -> [hd, j]
                    vhT_t = pc_vh.tile([128, DVT, LK], f32r, tag="vhT")
                    for mt in range(DVT):
                        for c in range(2):
                            ps = pc_ps.tile([128, 512], f32, tag="vh_ps")
                            for kt in range(DVT):
                                nc.tensor.matmul(
                                    ps[:],
                                    wv_t[:, kt, mt * 128:(mt + 1) * 128],
                                    vT_r[:, kt, c * 512:(c + 1) * 512],
                                    start=(kt == 0), stop=(kt == DVT - 1))
                            nc.scalar.copy(vhT_t[:, mt, c * 512:(c + 1) * 512], ps[:])

                    # W_h = vh^T.T @ fc_h -> [j, o], row-major to DRAM
                    wbuf = dram.tile([LK, DV], f32, tag="wbuf")
                    for jt in range(QT):
                        ps = pc_ps.tile([128, DV], f32, tag="w_ps")
                        for (o0, o1) in ((0, 512), (512, DV)):
                            for kt in range(DVT):
                                nc.tensor.matmul(
                                    ps[:, o0:o1],
                                    vhT_t[:, kt, jt * 128:(jt + 1) * 128],
                                    fc_t[:, kt, o0:o1],
                                    start=(kt == 0), stop=(kt == DVT - 1))
                        wev = pc_ev.tile([128, DV], f32, tag="wev")
                        nc.scalar.copy(wev[:], ps[:])
                        nc.gpsimd.dma_start(wbuf[jt * 128:(jt + 1) * 128, :], wev[:])

                    # gather W rows at j* and accumulate
                    gout = pc_g.tile([128, QT, DV], f32, tag="gout")
                    nc.gpsimd.dma_gather(
                        gout[:], wbuf[:],
                        idx_all[:, h * (LQ // 16):(h + 1) * (LQ // 16)],
                        num_idxs=LQ, num_idxs_reg=LQ, elem_size=DV)
                    gflat = gout[:].rearrange("p qt o -> p (qt o)")
                    if h == 0:
                        nc.vector.tensor_copy(part_t[:], gflat)
                    else:
                        nc.vector.tensor_add(part_t[:], part_t[:], gflat)

            nc.gpsimd.dma_start(part_ext[:],
                                part_t[:].rearrange("p (qt o) -> p qt o", qt=QT))

    nc.compile()
    return nc


def kernel(q, k, v, qpos, kpos, qk_mask, k_mask, post_softmax_mask,
           w_qs, w_ks, w_vs, fc):
    from concourse.bass_utils import run_bass_kernel_spmd

    if "nc" not in _compiled:
        _compiled["nc"] = _build_program()
    nc = _compiled["nc"]

    q = np.asarray(q, np.float32)
    k = np.asarray(k, np.float32)
    v = np.asarray(v, np.float32)
    qk_mask = np.asarray(qk_mask, np.float32)
    k_mask = np.asarray(k_mask, np.float32)
    post = np.asarray(post_softmax_mask, np.float32)
    w_qs = np.asarray(w_qs, np.float32)
    w_ks = np.asarray(w_ks, np.float32)
    w_vs = np.asarray(w_vs, np.float32)
    fc = np.asarray(fc, np.float32)

    wq8 = w_qs / np.float32(8.0)   # fold 1/sqrt(DK); exact power-of-2 scale

    in_maps = []
    for c in range(N_CORES):
        b, hh = c // 2, c % 2
        hs = slice(hh * HPC * DK, (hh + 1) * HPC * DK)
        vs = slice(hh * HPC * DV, (hh + 1) * HPC * DV)
        in_maps.append({
            "qT": np.ascontiguousarray(q[b].T),
            "kT": np.ascontiguousarray(k[b].T),
            "vT": np.ascontiguousarray(v[b].T),
            "wq": np.ascontiguousarray(wq8[:, hs]),
            "wk": np.ascontiguousarray(w_ks[:, hs]),
            "wv": np.ascontiguousarray(w_vs[:, vs]),
            "fc": np.ascontiguousarray(fc[vs, :]),
            "qk": np.ascontiguousarray(qk_mask[b, 0]),
            "post": np.ascontiguousarray(post[b, 0]),
            "km": np.ascontiguousarray(k_mask[b, 0, :, 0][None, :]),
        })

    res = run_bass_kernel_spmd(nc, in_maps, core_ids=list(range(N_CORES)))

    output = np.empty((B, LQ, DV), np.float32)
    attn = np.zeros((B, H, LQ, LK), np.float32)
    qidx = np.arange(LQ)
    for b in range(B):
        r0, r1 = res.results[2 * b], res.results[2 * b + 1]
        p0 = r0["part"].transpose(1, 0, 2).reshape(LQ, DV)
        p1 = r1["part"].transpose(1, 0, 2).reshape(LQ, DV)
        output[b] = (p0 + p1) + v[b]
        for hh, r in ((0, r0), (1, r1)):
            jt = r["jout"]  # [HPC, 128, QT], q = qt*128 + p
            for h in range(HPC):
                j = jt[h].T.reshape(LQ).astype(np.int64)  # [QT,128] -> q order
                attn[b, hh * HPC + h, qidx, j] = 1.0
    return output, attn


# revision 5
# speedup vs baseline: 2.0454x; 2.0454x over previous
"""Trainium2 Bass kernel for nn_MultiHeadAttention_515396076443 (sparse_attention).

Math shortcut that makes this fast: the reference applies straight-through
argmax hardening, `attn = hard - stop_gradient(attn) + attn`, right before
using `attn`.  In fp32 forward arithmetic `(0 - a) + a == 0` exactly and
`(1 - a) + a == 1` to within 1 ulp, so the effective attention matrix is a
one-hot selection of the top unmasked key per query:

    j*[b,h,q] = argmax_k ( qk_scores + qk_mask + k_mask  restricted to
                           post_softmax_mask == 1 )

(the top-k/softmax/renorm steps only rescale probabilities monotonically and
cannot change the argmax; non-top-k entries get exactly 0 probability, and
row selection commutes with the fc projection).  Then

    output[b,q] = sum_h (v[b] @ w_vs_h @ fc_h)[j*[b,h,q]] + v[b,q]
    attn[b,h,q] = one_hot(j*)      (argmax entry is 1 to within 1 ulp)

Sharding: 8 cores = 4 batches x 2 head-halves (6 heads each).
QK scores use native fp32 matmuls (argmax fidelity); the value path uses
float32r (~1.4e-4 rel err, 4x faster).
"""

import numpy as np

B, LQ, LK, H, DK, DV, E = 4, 1024, 1024, 12, 64, 768, 768
HPC = 6            # heads per core
QT = LQ // 128     # 8 q-tiles
ET = E // 128      # 6
DVT = DV // 128    # 6
MT = HPC * DK // 128  # 3 head-pair tiles
N_CORES = 8

_compiled = {}


def _build_program():
    from contextlib import ExitStack
    import concourse.tile as tile
    import concourse.mybir as mybir
    from concourse import bacc

    f32 = mybir.dt.float32
    f32r = mybir.dt.float32r
    u16 = mybir.dt.uint16
    i16 = mybir.dt.int16

    nc = bacc.Bacc(None, target_bir_lowering=False)

    qT_ext = nc.dram_tensor("qT", [E, LQ], f32, kind="ExternalInput")
    kT_ext = nc.dram_tensor("kT", [E, LK], f32, kind="ExternalInput")
    vT_ext = nc.dram_tensor("vT", [DV, LK], f32, kind="ExternalInput")
    wq_ext = nc.dram_tensor("wq", [E, HPC * DK], f32, kind="ExternalInput")
    wk_ext = nc.dram_tensor("wk", [E, HPC * DK], f32, kind="ExternalInput")
    wv_ext = nc.dram_tensor("wv", [DV, HPC * DV // 4], f32, kind="ExternalInput")
    fc_ext = nc.dram_tensor("fc", [HPC * DV // 4, DV], f32, kind="ExternalInput")
    qk_ext = nc.dram_tensor("qk", [LQ, LK], f32, kind="ExternalInput")
    post_ext = nc.dram_tensor("post", [LQ, LK], mybir.dt.uint8,
                              kind="ExternalInput")
    km_ext = nc.dram_tensor("km", [1, LK], f32, kind="ExternalInput")

    jout_ext = nc.dram_tensor("jout", [HPC, 128, QT], u16, kind="ExternalOutput")
    part_ext = nc.dram_tensor("part", [128, QT, DV], f32, kind="ExternalOutput")

    with tile.TileContext(nc) as tc:
        with ExitStack() as ctx:
            const = ctx.enter_context(tc.tile_pool(name="const", bufs=1))
            dram = ctx.enter_context(tc.tile_pool(name="dram", bufs=2, space="DRAM"))
            jdram = ctx.enter_context(tc.tile_pool(name="jdram", bufs=1, space="DRAM"))

            jtiles = const.tile([128, HPC, QT], u16, tag="jtiles")
            idx_all = const.tile([128, HPC * (LQ // 16)], i16, tag="idx_all")

            # gather the wv/fc head-half from per-core quarters (cores 2b+hh
            # for b=0..3 hold quarter b of head-half hh)
            wdram = ctx.enter_context(tc.tile_pool(name="wdram", bufs=1,
                                                   space="DRAM"))
            QW = HPC * DV // 4   # 1152
            wvq_i = wdram.tile([DV, QW], f32, tag="wvq_i")
            fcq_i = wdram.tile([QW, DV], f32, tag="fcq_i")
            wv_g = wdram.tile([4 * DV, QW], f32, tag="wv_g")
            fc_g = wdram.tile([HPC * DV, DV], f32, tag="fc_g")
            nc.gpsimd.dma_start(wvq_i[:], wv_ext[:])
            nc.gpsimd.dma_start(fcq_i[:], fc_ext[:])
            groups = [[0, 2, 4, 6], [1, 3, 5, 7]]
            nc.gpsimd.collective_compute(
                "AllGather", mybir.AluOpType.bypass, replica_groups=groups,
                ins=[wvq_i[:].opt()], outs=[wv_g[:].opt()])
            nc.gpsimd.collective_compute(
                "AllGather", mybir.AluOpType.bypass, replica_groups=groups,
                ins=[fcq_i[:].opt()], outs=[fc_g[:].opt()])

            with ExitStack() as ab:   # pools that live through phases A+B only
                abp = ab.enter_context(tc.tile_pool(name="abp", bufs=1))

                # ---------- constants ----------
                ones_t = abp.tile([1, 128], f32, tag="ones")
                nc.vector.memset(ones_t[:], 1.0)
                km_t = abp.tile([1, LK], f32, tag="km")
                nc.gpsimd.dma_start(km_t[:], km_ext[:])

                kb_t = abp.tile([128, LK], f32, tag="kb")
                with tc.tile_pool(name="kbp", bufs=1, space="PSUM") as kbp:
                    kb_ps = kbp.tile([128, LK], f32, tag="kb_ps")
                    for c in range(2):
                        nc.tensor.matmul(kb_ps[:, c * 512:(c + 1) * 512], ones_t[:],
                                         km_t[:, c * 512:(c + 1) * 512],
                                         start=True, stop=True)
                    nc.scalar.copy(kb_t[:], kb_ps[:])

                # ---------- phase A: qhT/khT projections (fp32) ----------
                qhT_t = abp.tile([128, MT, LQ], f32, tag="qhT")
                khT_t = abp.tile([128, MT, LK], f32, tag="khT")
                with tc.tile_pool(name="pa_sb", bufs=1) as pa_sb, \
                     tc.tile_pool(name="pa_ps", bufs=2, space="PSUM") as pa_ps:
                    wq_t = pa_sb.tile([128, ET, HPC * DK], f32, tag="wq")
                    wk_t = pa_sb.tile([128, ET, HPC * DK], f32, tag="wk")
                    qTs = pa_sb.tile([128, ET, LQ], f32, tag="qTs")
                    kTs = pa_sb.tile([128, ET, LK], f32, tag="kTs")
                    for kt in range(ET):
                        nc.gpsimd.dma_start(wq_t[:, kt, :],
                                            wq_ext[kt * 128:(kt + 1) * 128, :])
                        nc.gpsimd.dma_start(wk_t[:, kt, :],
                                            wk_ext[kt * 128:(kt + 1) * 128, :])
                        nc.gpsimd.dma_start(qTs[:, kt, :],
                                            qT_ext[kt * 128:(kt + 1) * 128, :])
                        nc.gpsimd.dma_start(kTs[:, kt, :],
                                            kT_ext[kt * 128:(kt + 1) * 128, :])
                    for (w_t, src, dst) in ((wq_t, qTs, qhT_t), (wk_t, kTs, khT_t)):
                        for mt in range(MT):
                            for c in range(2):
                                ps = pa_ps.tile([128, 512], f32, tag="pa")
                                for kt in range(ET):
                                    nc.tensor.matmul(
                                        ps[:],
                                        w_t[:, kt, mt * 128:(mt + 1) * 128],
                                        src[:, kt, c * 512:(c + 1) * 512],
                                        start=(kt == 0), stop=(kt == ET - 1))
                                nc.scalar.copy(dst[:, mt, c * 512:(c + 1) * 512],
                                               ps[:])

                # ---------- phase B: scores + argmax (fp32) ----------
                with tc.tile_pool(name="pb_sb", bufs=2) as pb_sb, \
                     tc.tile_pool(name="pb_s", bufs=3) as pb_s, \
                     tc.tile_pool(name="pb_ps", bufs=4, space="PSUM") as pb_ps:
                    for qt in range(QT):
                        qk_t = pb_sb.tile([128, LK], f32, tag="qk")
                        post_t = pb_sb.tile([128, LK], f32, tag="post")
                        nc.gpsimd.dma_start(qk_t[:], qk_ext[qt * 128:(qt + 1) * 128, :])
                        nc.gpsimd.dma_start(post_t[:],
                                            post_ext[qt * 128:(qt + 1) * 128, :])
                        pen_t = pb_sb.tile([128, LK], f32, tag="pen")
                        nc.scalar.activation(pen_t[:], post_t[:],
                                             mybir.ActivationFunctionType.Copy,
                                             bias=-1e9, scale=1e9)
                        comb_t = pb_sb.tile([128, LK], f32, tag="comb")
                        nc.vector.tensor_add(comb_t[:], qk_t[:], pen_t[:])
                        nc.vector.tensor_add(comb_t[:], comb_t[:], kb_t[:])

                        for hp in range(MT):  # head pairs, packed on row groups
                            accs = [pb_ps.tile([128, LK], f32, tag="acc",
                                               name=f"acc{i}")
                                    for i in range(2)]
                            for c in range(2):
                                for hi in range(2):
                                    nc.tensor.matmul(
                                        accs[hi][:, c * 512:(c + 1) * 512],
                                        qhT_t[64 * hi:64 * (hi + 1), hp,
                                              qt * 128:(qt + 1) * 128],
                                        khT_t[64 * hi:64 * (hi + 1), hp,
                                              c * 512:(c + 1) * 512],
                                        start=True, stop=True)
                            for hi in range(2):
                                h = 2 * hp + hi
                                s_t = pb_s.tile([128, LK], f32, tag="s")
                                nc.vector.tensor_add(s_t[:], accs[hi][:], comb_t[:])
                                max8 = pb_s.tile([128, 8], f32, tag="max8")
                                idx8 = pb_s.tile([128, 8], u16, tag="idx8")
                                nc.vector.max(max8[:], s_t[:])
                                nc.vector.max_index(idx8[:], max8[:], s_t[:])
                                nc.vector.tensor_copy(jtiles[:, h, qt:qt + 1],
                                                      idx8[:, 0:1])

            # j* to DRAM in q-order flat layout, then read back in the gather
            # ucode's index layout (idx i at partition i%16, col i//16),
            # replicated into all 8 partition groups.
            jscr = jdram.tile([HPC, LQ], u16, tag="jscr")
            for h in range(HPC):
                nc.gpsimd.dma_start(jout_ext[h], jtiles[:, h, :])
                nc.gpsimd.dma_start(
                    jscr[h].rearrange("(c p) -> p c", p=128),
                    jtiles[:, h, :])
            for g in range(8):
                nc.gpsimd.dma_start(
                    idx_all[16 * g:16 * (g + 1), :],
                    jscr[:].rearrange("h (c p) -> p (h c)", p=16))

            # ---------- phase C: value path (float32r) + gather ----------
            vT_r = const.tile([128, DVT, LK], f32r, tag="vT_r")
            for kt in range(DVT):
                nc.gpsimd.dma_start(vT_r[:, kt, :], vT_ext[kt * 128:(kt + 1) * 128, :])

            part_t = const.tile([128, QT * DV], f32, tag="part")
            with tc.tile_pool(name="pc_w", bufs=2) as pc_w, \
                 tc.tile_pool(name="pc_vh", bufs=1) as pc_vh, \
                 tc.tile_pool(name="pc_ev", bufs=2) as pc_ev, \
                 tc.tile_pool(name="pc_ps", bufs=2, space="PSUM") as pc_ps, \
                 tc.tile_pool(name="pc_g", bufs=1) as pc_g:
                for h in range(HPC):
                    wv_t = pc_w.tile([128, DVT, DV], f32r, tag="wv")
                    fc_t = pc_w.tile([128, DVT, DV], f32r, tag="fc")
                    for kt in range(DVT):
                        # wv_g rows [768*qu + dv], cols local to quarter qu
                        done = 0
                        while done < DV:
                            gcol = h * DV + done
                            qu, off = divmod(gcol, QW)
                            seg = min(QW - off, DV - done)
                            nc.gpsimd.dma_start(
                                wv_t[:, kt, done:done + seg],
                                wv_g[DV * qu + kt * 128:DV * qu + (kt + 1) * 128,
                                     off:off + seg])
                            done += seg
                        nc.gpsimd.dma_start(
                            fc_t[:, kt, :],
                            fc_g[h * DV + kt * 128:h * DV + (kt + 1) * 128, :])

                    # vh^T = wv_h.T @ vT  -> [hd, j]
                    vhT_t = pc_vh.tile([128, DVT, LK], f32r, tag="vhT")
                    for mt in range(DVT):
                        for c in range(2):
                            ps = pc_ps.tile([128, 512], f32, tag="vh_ps")
                            for kt in range(DVT):
                                nc.tensor.matmul(
                                    ps[:],
                                    wv_t[:, kt, mt * 128:(mt + 1) * 128],
                                    vT_r[:, kt, c * 512:(c + 1) * 512],
                                    start=(kt == 0), stop=(kt == DVT - 1))
                            nc.scalar.copy(vhT_t[:, mt, c * 512:(c + 1) * 512], ps[:])

                    # W_h = vh^T.T @ fc_h -> [j, o], row-major to DRAM
                    wbuf = dram.tile([LK, DV], f32, tag="wbuf")
                    for jt in range(QT):
                        ps = pc_ps.tile([128, DV], f32, tag="w_ps")
                        for (o0, o1) in ((0, 512), (512, DV)):
                            for kt in range(DVT):
                                nc.tensor.matmul(
                                    ps[:, o0:o1],
                                    vhT_t[:, kt, jt * 128:(jt + 1) * 128],
                                    fc_t[:, kt, o0:o1],
                                    start=(kt == 0), stop=(kt == DVT - 1))
                        wev = pc_ev.tile([128, DV], f32, tag="wev")
                        nc.scalar.copy(wev[:], ps[:])
                        nc.gpsimd.dma_start(wbuf[jt * 128:(jt + 1) * 128, :], wev[:])

                    # gather W rows at j* and accumulate
                    gout = pc_g.tile([128, QT, DV], f32, tag="gout")
                    nc.gpsimd.dma_gather(
                        gout[:], wbuf[:],
                        idx_all[:, h * (LQ // 16):(h + 1) * (LQ // 16)],
                        num_idxs=LQ, num_idxs_reg=LQ, elem_size=DV)
                    gflat = gout[:].rearrange("p qt o -> p (qt o)")
                    if h == 0:
                        nc.vector.tensor_copy(part_t[:], gflat)
                    else:
                        nc.vector.tensor_add(part_t[:], part_t[:], gflat)

            nc.gpsimd.dma_start(part_ext[:],
                                part_t[:].rearrange("p (qt o) -> p qt o", qt=QT))

    nc.compile()
    return nc


def kernel(q, k, v, qpos, kpos, qk_mask, k_mask, post_softmax_mask,
           w_qs, w_ks, w_vs, fc):
    from concourse.bass_utils import run_bass_kernel_spmd

    if "nc" not in _compiled:
        _compiled["nc"] = _build_program()
    nc = _compiled["nc"]

    q = np.asarray(q, np.float32)
    k = np.asarray(k, np.float32)
    v = np.asarray(v, np.float32)
    qk_mask = np.asarray(qk_mask, np.float32)
    k_mask = np.asarray(k_mask, np.float32)
    post = np.asarray(post_softmax_mask, np.float32)
    w_qs = np.asarray(w_qs, np.float32)
    w_ks = np.asarray(w_ks, np.float32)
    w_vs = np.asarray(w_vs, np.float32)
    fc = np.asarray(fc, np.float32)

    wq8 = w_qs / np.float32(8.0)   # fold 1/sqrt(DK); exact power-of-2 scale

    in_maps = []
    for c in range(N_CORES):
        b, hh = c // 2, c % 2
        hs = slice(hh * HPC * DK, (hh + 1) * HPC * DK)
        QW = HPC * DV // 4
        qs = slice(hh * HPC * DV + b * QW, hh * HPC * DV + (b + 1) * QW)
        in_maps.append({
            "qT": np.ascontiguousarray(q[b].T),
            "kT": np.ascontiguousarray(k[b].T),
            "vT": np.ascontiguousarray(v[b].T),
            "wq": np.ascontiguousarray(wq8[:, hs]),
            "wk": np.ascontiguousarray(w_ks[:, hs]),
            "wv": np.ascontiguousarray(w_vs[:, qs]),
            "fc": np.ascontiguousarray(fc[qs, :]),
            "qk": np.ascontiguousarray(qk_mask[b, 0]),
            "post": post[b, 0].astype(np.uint8),
            "km": np.ascontiguousarray(k_mask[b, 0, :, 0][None, :]),
        })

    res = run_bass_kernel_spmd(nc, in_maps, core_ids=list(range(N_CORES)))

    output = np.empty((B, LQ, DV), np.float32)
    attn = np.zeros((B, H, LQ, LK), np.float32)
    qidx = np.arange(LQ)
    for b in range(B):
        r0, r1 = res.results[2 * b], res.results[2 * b + 1]
        p0 = r0["part"].transpose(1, 0, 2).reshape(LQ, DV)
        p1 = r1["part"].transpose(1, 0, 2).reshape(LQ, DV)
        output[b] = (p0 + p1) + v[b]
        for hh, r in ((0, r0), (1, r1)):
            jt = r["jout"]  # [HPC, 128, QT], q = qt*128 + p
            for h in range(HPC):
                j = jt[h].T.reshape(LQ).astype(np.int64)  # [QT,128] -> q order
                attn[b, hh * HPC + h, qidx, j] = 1.0
    return output, attn


# revision 9
# speedup vs baseline: 2.4127x; 1.1795x over previous
"""Trainium2 Bass kernel for nn_MultiHeadAttention_515396076443 (sparse_attention).

Math shortcut that makes this fast: the reference applies straight-through
argmax hardening, `attn = hard - stop_gradient(attn) + attn`, right before
using `attn`.  In fp32 forward arithmetic `(0 - a) + a == 0` exactly and
`(1 - a) + a == 1` to within 1 ulp, so the effective attention matrix is a
one-hot selection of the top unmasked key per query:

    j*[b,h,q] = argmax_k ( qk_scores + qk_mask + k_mask  restricted to
                           post_softmax_mask == 1 )

(the top-k/softmax/renorm steps only rescale probabilities monotonically and
cannot change the argmax; non-top-k entries get exactly 0 probability, and
row selection commutes with the fc projection).  Then

    output[b,q] = sum_h (v[b] @ w_vs_h @ fc_h)[j*[b,h,q]] + v[b,q]
    attn[b,h,q] = one_hot(j*)      (argmax entry is 1 to within 1 ulp)

Sharding: 8 cores = 4 batches x 2 head-halves (6 heads each).
QK scores use native fp32 matmuls (argmax fidelity); the value path uses
float32r (~1.4e-4 rel err, 4x faster).
"""

import numpy as np

B, LQ, LK, H, DK, DV, E = 4, 1024, 1024, 12, 64, 768, 768
HPC = 6            # heads per core
QT = LQ // 128     # 8 q-tiles
ET = E // 128      # 6
DVT = DV // 128    # 6
MT = HPC * DK // 128  # 3 head-pair tiles
N_CORES = 8

_compiled = {}


def _build_program():
    from contextlib import ExitStack
    import concourse.tile as tile
    import concourse.mybir as mybir
    from concourse import bacc

    f32 = mybir.dt.float32
    f32r = mybir.dt.float32r
    u16 = mybir.dt.uint16
    i16 = mybir.dt.int16

    nc = bacc.Bacc(None, target_bir_lowering=False)

    qTh_ext = nc.dram_tensor("qTh", [E // 2, LQ], f32, kind="ExternalInput")
    kTh_ext = nc.dram_tensor("kTh", [E // 2, LK], f32, kind="ExternalInput")
    vTh_ext = nc.dram_tensor("vTh", [DV // 2, LK], f32, kind="ExternalInput")
    wq_ext = nc.dram_tensor("wq", [E, HPC * DK], f32, kind="ExternalInput")
    wk_ext = nc.dram_tensor("wk", [E, HPC * DK], f32, kind="ExternalInput")
    wv_ext = nc.dram_tensor("wv", [DV, H * DV // 8], f32, kind="ExternalInput")
    fc_ext = nc.dram_tensor("fc", [H * DV // 8, DV], f32, kind="ExternalInput")
    qkh_ext = nc.dram_tensor("qkh", [LQ // 2, LK], f32, kind="ExternalInput")
    posth_ext = nc.dram_tensor("posth", [LQ // 2, LK], mybir.dt.uint8,
                               kind="ExternalInput")
    km_ext = nc.dram_tensor("km", [1, LK], f32, kind="ExternalInput")

    jout_ext = nc.dram_tensor("jout", [HPC, 128, QT], u16, kind="ExternalOutput")
    part_ext = nc.dram_tensor("part", [128, QT, DV], f32, kind="ExternalOutput")

    with tile.TileContext(nc) as tc:
        with ExitStack() as ctx:
            const = ctx.enter_context(tc.tile_pool(name="const", bufs=1))
            dram = ctx.enter_context(tc.tile_pool(name="dram", bufs=2, space="DRAM"))
            jdram = ctx.enter_context(tc.tile_pool(name="jdram", bufs=1, space="DRAM"))

            jtiles = const.tile([128, HPC, QT], u16, tag="jtiles")
            idx_all = const.tile([128, HPC * (LQ // 16)], i16, tag="idx_all")

            # input shards are re-assembled on device:
            #  - wv/fc: head-half quarters, AllGather over the 4 cores
            #    sharing the head-half
            #  - qT/kT/vT/qk/post: halves, AllGather over batch pairs
            wdram = ctx.enter_context(tc.tile_pool(name="wdram", bufs=1,
                                                   space="DRAM"))
            QW = H * DV // 8   # 1152
            wvq_i = wdram.tile([DV, QW], f32, tag="wvq_i")
            fcq_i = wdram.tile([QW, DV], f32, tag="fcq_i")
            wv_g = wdram.tile([4 * DV, QW], f32, tag="wv_g")
            fc_g = wdram.tile([HPC * DV, DV], f32, tag="fc_g")
            nc.gpsimd.dma_start(wvq_i[:], wv_ext[:])
            nc.gpsimd.dma_start(fcq_i[:], fc_ext[:])
            halves = [[0, 2, 4, 6], [1, 3, 5, 7]]
            pairs = [[0, 1], [2, 3], [4, 5], [6, 7]]
            nc.gpsimd.collective_compute(
                "AllGather", mybir.AluOpType.bypass, replica_groups=halves,
                ins=[wvq_i[:].opt()], outs=[wv_g[:].opt()])
            nc.gpsimd.collective_compute(
                "AllGather", mybir.AluOpType.bypass, replica_groups=halves,
                ins=[fcq_i[:].opt()], outs=[fc_g[:].opt()])

            qTh_i = wdram.tile([E // 2, LQ], f32, tag="qTh_i")
            kTh_i = wdram.tile([E // 2, LK], f32, tag="kTh_i")
            vTh_i = wdram.tile([DV // 2, LK], f32, tag="vTh_i")
            qkh_i = wdram.tile([LQ // 2, LK], f32, tag="qkh_i")
            posth_i = wdram.tile([LQ // 2, LK], mybir.dt.uint8, tag="posth_i")
            qT_g = wdram.tile([E, LQ], f32, tag="qT_g")
            kT_g = wdram.tile([E, LK], f32, tag="kT_g")
            vT_g = wdram.tile([DV, LK], f32, tag="vT_g")
            qk_g = wdram.tile([LQ, LK], f32, tag="qk_g")
            post_g = wdram.tile([LQ, LK], mybir.dt.uint8, tag="post_g")
            nc.gpsimd.dma_start(qTh_i[:], qTh_ext[:])
            nc.gpsimd.dma_start(kTh_i[:], kTh_ext[:])
            nc.gpsimd.dma_start(vTh_i[:], vTh_ext[:])
            nc.gpsimd.dma_start(qkh_i[:], qkh_ext[:])
            nc.gpsimd.dma_start(posth_i[:], posth_ext[:])
            for (i_t, g_t) in ((qTh_i, qT_g), (kTh_i, kT_g), (vTh_i, vT_g),
                               (qkh_i, qk_g), (posth_i, post_g)):
                nc.gpsimd.collective_compute(
                    "AllGather", mybir.AluOpType.bypass, replica_groups=pairs,
                    ins=[i_t[:].opt()], outs=[g_t[:].opt()])

            with ExitStack() as ab:   # pools that live through phases A+B only
                abp = ab.enter_context(tc.tile_pool(name="abp", bufs=1))

                # ---------- constants ----------
                ones_t = abp.tile([1, 128], f32, tag="ones")
                nc.vector.memset(ones_t[:], 1.0)
                km_t = abp.tile([1, LK], f32, tag="km")
                nc.gpsimd.dma_start(km_t[:], km_ext[:])

                kb_t = abp.tile([128, LK], f32, tag="kb")
                with tc.tile_pool(name="kbp", bufs=1, space="PSUM") as kbp:
                    kb_ps = kbp.tile([128, LK], f32, tag="kb_ps")
                    for c in range(2):
                        nc.tensor.matmul(kb_ps[:, c * 512:(c + 1) * 512], ones_t[:],
                                         km_t[:, c * 512:(c + 1) * 512],
                                         start=True, stop=True)
                    nc.scalar.copy(kb_t[:], kb_ps[:])

                # ---------- phase A: qhT/khT projections (fp32) ----------
                qhT_t = abp.tile([128, MT, LQ], f32, tag="qhT")
                khT_t = abp.tile([128, MT, LK], f32, tag="khT")
                with tc.tile_pool(name="pa_sb", bufs=1) as pa_sb, \
                     tc.tile_pool(name="pa_ps", bufs=2, space="PSUM") as pa_ps:
                    wq_t = pa_sb.tile([128, ET, HPC * DK], f32, tag="wq")
                    wk_t = pa_sb.tile([128, ET, HPC * DK], f32, tag="wk")
                    qTs = pa_sb.tile([128, ET, LQ], f32, tag="qTs")
                    kTs = pa_sb.tile([128, ET, LK], f32, tag="kTs")
                    for kt in range(ET):
                        nc.gpsimd.dma_start(wq_t[:, kt, :],
                                            wq_ext[kt * 128:(kt + 1) * 128, :])
                        nc.gpsimd.dma_start(wk_t[:, kt, :],
                                            wk_ext[kt * 128:(kt + 1) * 128, :])
                        nc.gpsimd.dma_start(qTs[:, kt, :],
                                            qT_g[kt * 128:(kt + 1) * 128, :])
                        nc.gpsimd.dma_start(kTs[:, kt, :],
                                            kT_g[kt * 128:(kt + 1) * 128, :])
                    for (w_t, src, dst) in ((wq_t, qTs, qhT_t), (wk_t, kTs, khT_t)):
                        for mt in range(MT):
                            for c in range(2):
                                ps = pa_ps.tile([128, 512], f32, tag="pa")
                                for kt in range(ET):
                                    nc.tensor.matmul(
                                        ps[:],
                                        w_t[:, kt, mt * 128:(mt + 1) * 128],
                                        src[:, kt, c * 512:(c + 1) * 512],
                                        start=(kt == 0), stop=(kt == ET - 1))
                                nc.scalar.copy(dst[:, mt, c * 512:(c + 1) * 512],
                                               ps[:])

                # ---------- phase B: scores + argmax (fp32) ----------
                with tc.tile_pool(name="pb_sb", bufs=2) as pb_sb, \
                     tc.tile_pool(name="pb_s", bufs=3) as pb_s, \
                     tc.tile_pool(name="pb_ps", bufs=4, space="PSUM") as pb_ps:
                    for qt in range(QT):
                        qk_t = pb_sb.tile([128, LK], f32, tag="qk")
                        post_t = pb_sb.tile([128, LK], f32, tag="post")
                        nc.gpsimd.dma_start(qk_t[:], qk_g[qt * 128:(qt + 1) * 128, :])
                        nc.gpsimd.dma_start(post_t[:],
                                            post_g[qt * 128:(qt + 1) * 128, :])
                        pen_t = pb_sb.tile([128, LK], f32, tag="pen")
                        nc.scalar.activation(pen_t[:], post_t[:],
                                             mybir.ActivationFunctionType.Copy,
                                             bias=-1e9, scale=1e9)
                        comb_t = pb_sb.tile([128, LK], f32, tag="comb")
                        nc.vector.tensor_add(comb_t[:], qk_t[:], pen_t[:])
                        nc.vector.tensor_add(comb_t[:], comb_t[:], kb_t[:])

                        for hp in range(MT):  # head pairs, packed on row groups
                            accs = [pb_ps.tile([128, LK], f32, tag="acc",
                                               name=f"acc{i}")
                                    for i in range(2)]
                            for c in range(2):
                                for hi in range(2):
                                    nc.tensor.matmul(
                                        accs[hi][:, c * 512:(c + 1) * 512],
                                        qhT_t[64 * hi:64 * (hi + 1), hp,
                                              qt * 128:(qt + 1) * 128],
                                        khT_t[64 * hi:64 * (hi + 1), hp,
                                              c * 512:(c + 1) * 512],
                                        start=True, stop=True)
                            for hi in range(2):
                                h = 2 * hp + hi
                                s_t = pb_s.tile([128, LK], f32, tag="s")
                                nc.vector.tensor_add(s_t[:], accs[hi][:], comb_t[:])
                                max8 = pb_s.tile([128, 8], f32, tag="max8")
                                idx8 = pb_s.tile([128, 8], u16, tag="idx8")
                                nc.vector.max(max8[:], s_t[:])
                                nc.vector.max_index(idx8[:], max8[:], s_t[:])
                                nc.vector.tensor_copy(jtiles[:, h, qt:qt + 1],
                                                      idx8[:, 0:1])

            # j* to DRAM in q-order flat layout, then read back in the gather
            # ucode's index layout (idx i at partition i%16, col i//16),
            # replicated into all 8 partition groups.
            jscr = jdram.tile([HPC, LQ], u16, tag="jscr")
            for h in range(HPC):
                nc.gpsimd.dma_start(jout_ext[h], jtiles[:, h, :])
                nc.gpsimd.dma_start(
                    jscr[h].rearrange("(c p) -> p c", p=128),
                    jtiles[:, h, :])
            for g in range(8):
                nc.gpsimd.dma_start(
                    idx_all[16 * g:16 * (g + 1), :],
                    jscr[:].rearrange("h (c p) -> p (h c)", p=16))

            # ---------- phase C: value path (float32r) + gather ----------
            vT_r = const.tile([128, DVT, LK], f32r, tag="vT_r")
            for kt in range(DVT):
                nc.gpsimd.dma_start(vT_r[:, kt, :], vT_g[kt * 128:(kt + 1) * 128, :])

            part_t = const.tile([128, QT * DV], f32, tag="part")
            with tc.tile_pool(name="pc_w", bufs=2) as pc_w, \
                 tc.tile_pool(name="pc_vh", bufs=1) as pc_vh, \
                 tc.tile_pool(name="pc_ev", bufs=2) as pc_ev, \
                 tc.tile_pool(name="pc_ps", bufs=2, space="PSUM") as pc_ps, \
                 tc.tile_pool(name="pc_g", bufs=1) as pc_g:
                for h in range(HPC):
                    wv_t = pc_w.tile([128, DVT, DV], f32r, tag="wv")
                    fc_t = pc_w.tile([128, DVT, DV], f32r, tag="fc")
                    for kt in range(DVT):
                        # wv_g rows [768*qu + dv], cols local to quarter qu
                        done = 0
                        while done < DV:
                            gcol = h * DV + done
                            qu, off = divmod(gcol, QW)
                            seg = min(QW - off, DV - done)
                            nc.gpsimd.dma_start(
                                wv_t[:, kt, done:done + seg],
                                wv_g[DV * qu + kt * 128:DV * qu + (kt + 1) * 128,
                                     off:off + seg])
                            done += seg
                        nc.gpsimd.dma_start(
                            fc_t[:, kt, :],
                            fc_g[h * DV + kt * 128:h * DV + (kt + 1) * 128, :])

                    # vh^T = wv_h.T @ vT  -> [hd, j]
                    vhT_t = pc_vh.tile([128, DVT, LK], f32r, tag="vhT")
                    for mt in range(DVT):
                        for c in range(2):
                            ps = pc_ps.tile([128, 512], f32, tag="vh_ps")
                            for kt in range(DVT):
                                nc.tensor.matmul(
                                    ps[:],
                                    wv_t[:, kt, mt * 128:(mt + 1) * 128],
                                    vT_r[:, kt, c * 512:(c + 1) * 512],
                                    start=(kt == 0), stop=(kt == DVT - 1))
                            nc.scalar.copy(vhT_t[:, mt, c * 512:(c + 1) * 512], ps[:])

                    # W_h = vh^T.T @ fc_h -> [j, o], row-major to DRAM
                    wbuf = dram.tile([LK, DV], f32, tag="wbuf")
                    for jt in range(QT):
                        ps = pc_ps.tile([128, DV], f32, tag="w_ps")
                        for (o0, o1) in ((0, 512), (512, DV)):
                            for kt in range(DVT):
                                nc.tensor.matmul(
                                    ps[:, o0:o1],
                                    vhT_t[:, kt, jt * 128:(jt + 1) * 128],
                                    fc_t[:, kt, o0:o1],
                                    start=(kt == 0), stop=(kt == DVT - 1))
                        wev = pc_ev.tile([128, DV], f32, tag="wev")
                        nc.scalar.copy(wev[:], ps[:])
                        nc.gpsimd.dma_start(wbuf[jt * 128:(jt + 1) * 128, :], wev[:])

                    # gather W rows at j* and accumulate
                    gout = pc_g.tile([128, QT, DV], f32, tag="gout")
                    nc.gpsimd.dma_gather(
                        gout[:], wbuf[:],
                        idx_all[:, h * (LQ // 16):(h + 1) * (LQ // 16)],
                        num_idxs=LQ, num_idxs_reg=LQ, elem_size=DV)
                    gflat = gout[:].rearrange("p qt o -> p (qt o)")
                    if h == 0:
                        nc.vector.tensor_copy(part_t[:], gflat)
                    else:
                        nc.vector.tensor_add(part_t[:], part_t[:], gflat)

            nc.gpsimd.dma_start(part_ext[:],
                                part_t[:].rearrange("p (qt o) -> p qt o", qt=QT))

    nc.compile()
    return nc


def _make_in_maps(q, k, v, qk_mask, k_mask, post, w_qs, w_ks, w_vs, fc):
    wq8 = w_qs / np.float32(8.0)   # fold 1/sqrt(DK); exact power-of-2 scale
    in_maps = []
    for c in range(N_CORES):
        b, hh = c // 2, c % 2
        hs = slice(hh * HPC * DK, (hh + 1) * HPC * DK)
        QW = HPC * DV // 4
        qs = slice(hh * HPC * DV + b * QW, hh * HPC * DV + (b + 1) * QW)
        eh = slice(hh * E // 2, (hh + 1) * E // 2)
        qh2 = slice(hh * LQ // 2, (hh + 1) * LQ // 2)
        in_maps.append({
            "qTh": np.ascontiguousarray(q[b].T[eh]),
            "kTh": np.ascontiguousarray(k[b].T[eh]),
            "vTh": np.ascontiguousarray(v[b].T[eh]),
            "wq": np.ascontiguousarray(wq8[:, hs]),
            "wk": np.ascontiguousarray(w_ks[:, hs]),
            "wv": np.ascontiguousarray(w_vs[:, qs]),
            "fc": np.ascontiguousarray(fc[qs, :]),
            "qkh": np.ascontiguousarray(qk_mask[b, 0, qh2]),
            "posth": post[b, 0, qh2].astype(np.uint8),
            "km": np.ascontiguousarray(k_mask[b, 0, :, 0][None, :]),
        })
    return in_maps


def kernel(q, k, v, qpos, kpos, qk_mask, k_mask, post_softmax_mask,
           w_qs, w_ks, w_vs, fc):
    from concourse.bass_utils import run_bass_kernel_spmd

    if "nc" not in _compiled:
        _compiled["nc"] = _build_program()
    nc = _compiled["nc"]

    q = np.asarray(q, np.float32)
    k = np.asarray(k, np.float32)
    v = np.asarray(v, np.float32)
    qk_mask = np.asarray(qk_mask, np.float32)
    k_mask = np.asarray(k_mask, np.float32)
    post = np.asarray(post_softmax_mask, np.float32)
    w_qs = np.asarray(w_qs, np.float32)
    w_ks = np.asarray(w_ks, np.float32)
    w_vs = np.asarray(w_vs, np.float32)
    fc = np.asarray(fc, np.float32)

    in_maps = _make_in_maps(q, k, v, qk_mask, k_mask, post, w_qs, w_ks, w_vs, fc)

    res = run_bass_kernel_spmd(nc, in_maps, core_ids=list(range(N_CORES)))

    output = np.empty((B, LQ, DV), np.float32)
    attn = np.zeros((B, H, LQ, LK), np.float32)
    qidx = np.arange(LQ)
    for b in range(B):
        r0, r1 = res.results[2 * b], res.results[2 * b + 1]
        p0 = r0["part"].transpose(1, 0, 2).reshape(LQ, DV)
        p1 = r1["part"].transpose(1, 0, 2).reshape(LQ, DV)
        output[b] = (p0 + p1) + v[b]
        for hh, r in ((0, r0), (1, r1)):
            jt = r["jout"]  # [HPC, 128, QT], q = qt*128 + p
            for h in range(HPC):
                j = jt[h].T.reshape(LQ).astype(np.int64)  # [QT,128] -> q order
                attn[b, hh * HPC + h, qidx, j] = 1.0
    return output, attn


# revision 18
# speedup vs baseline: 3.9946x; 1.6556x over previous
"""Trainium2 Bass kernel for nn_MultiHeadAttention_515396076443 (sparse_attention).

Math shortcut that makes this fast: the reference applies straight-through
argmax hardening, `attn = hard - stop_gradient(attn) + attn`, right before
using `attn`.  In fp32 forward arithmetic `(0 - a) + a == 0` exactly and
`(1 - a) + a == 1` to within 1 ulp, so the effective attention matrix is a
one-hot selection of the top unmasked key per query:

    j*[b,h,q] = argmax_k ( qk_scores + qk_mask + k_mask  restricted to
                           post_softmax_mask == 1 )

(the top-k/softmax/renorm steps only rescale probabilities monotonically and
cannot change the argmax; non-top-k entries get exactly 0 probability, and
row selection commutes with the fc projection).  Then

    output[b,q] = sum_h (v[b] @ w_vs_h @ fc_h)[j*[b,h,q]] + v[b,q]
    attn[b,h,q] = one_hot(j*)      (argmax entry is 1 to within 1 ulp)

Sharding: 8 cores = 4 batches x 2 head-halves (6 heads each).
QK scores use native fp32 matmuls (argmax fidelity); the value path uses
float32r (~1.4e-4 rel err, 4x faster).
"""

import numpy as np

B, LQ, LK, H, DK, DV, E = 4, 1024, 1024, 12, 64, 768, 768
HPC = 6            # heads per core
QT = LQ // 128     # 8 q-tiles
ET = E // 128      # 6
DVT = DV // 128    # 6
MT = HPC * DK // 128  # 3 head-pair tiles
N_CORES = 8

_compiled = {}


def _build_program():
    from contextlib import ExitStack
    import concourse.tile as tile
    import concourse.mybir as mybir
    from concourse import bacc

    f32 = mybir.dt.float32
    f32r = mybir.dt.float32r
    u16 = mybir.dt.uint16
    i16 = mybir.dt.int16

    nc = bacc.Bacc(None, target_bir_lowering=False)

    PB = (3 * (E // 2) * LQ * 4) + (LQ // 2) * LK * 4 + (LQ // 2) * LK  # 7340032
    pair_ext = nc.dram_tensor("pairb", [PB], mybir.dt.uint8,
                              kind="ExternalInput")
    wq_ext = nc.dram_tensor("wq", [E, HPC * DK // 4], f32, kind="ExternalInput")
    wk_ext = nc.dram_tensor("wk", [E, HPC * DK // 4], f32, kind="ExternalInput")
    f16 = mybir.dt.float16
    wv_ext = nc.dram_tensor("wv", [DV, H * DV // 8], f16, kind="ExternalInput")
    fc_ext = nc.dram_tensor("fc", [H * DV // 8, DV], f16, kind="ExternalInput")
    km_ext = nc.dram_tensor("km", [1, LK], f32, kind="ExternalInput")

    jout_ext = nc.dram_tensor("jout", [128, HPC, QT], u16, kind="ExternalOutput")
    part_ext = nc.dram_tensor("part", [128, QT, DV], f32, kind="ExternalOutput")

    with tile.TileContext(nc) as tc:
        with ExitStack() as ctx:
            const = ctx.enter_context(tc.tile_pool(name="const", bufs=1))
            dram = ctx.enter_context(tc.tile_pool(name="dram", bufs=2, space="DRAM"))
            jdram = ctx.enter_context(tc.tile_pool(name="jdram", bufs=1, space="DRAM"))

            jtiles = const.tile([128, HPC, QT], u16, tag="jtiles")
            idx_all = const.tile([128, HPC * (LQ // 16)], i16, tag="idx_all")

            # input shards are re-assembled on device:
            #  - wv/fc: head-half quarters, AllGather over the 4 cores
            #    sharing the head-half
            #  - qT/kT/vT/qk/post: halves, AllGather over batch pairs
            wdram = ctx.enter_context(tc.tile_pool(name="wdram", bufs=1,
                                                   space="DRAM"))
            QW = H * DV // 8   # 1152
            QWK = HPC * DK // 4  # 96
            halves = [[0, 2, 4, 6], [1, 3, 5, 7]]
            pairs = [[0, 1], [2, 3], [4, 5], [6, 7]]

            wvq_i = wdram.tile([DV, QW], f16, tag="wvq_i")
            fcq_i = wdram.tile([QW, DV], f16, tag="fcq_i")
            wqq_i = wdram.tile([E, QWK], f32, tag="wqq_i")
            wkq_i = wdram.tile([E, QWK], f32, tag="wkq_i")
            wv_g = wdram.tile([4 * DV, QW], f16, tag="wv_g")
            fc_g = wdram.tile([HPC * DV, DV], f16, tag="fc_g")
            wq_g = wdram.tile([4 * E, QWK], f32, tag="wq_g")
            wk_g = wdram.tile([4 * E, QWK], f32, tag="wk_g")
            nc.gpsimd.dma_start(wvq_i[:], wv_ext[:])
            nc.gpsimd.dma_start(fcq_i[:], fc_ext[:])
            nc.gpsimd.dma_start(wqq_i[:], wq_ext[:])
            nc.gpsimd.dma_start(wkq_i[:], wk_ext[:])

            pair_i = wdram.tile([PB], mybir.dt.uint8, tag="pair_i")
            pair_g = wdram.tile([2 * PB], mybir.dt.uint8, tag="pair_g")
            nc.gpsimd.dma_start(pair_i[:], pair_ext[:])

            # byte offsets inside each packed half
            O_QT, O_KT, O_VT = 0, 1572864, 3145728
            O_QK, O_PO = 4718592, 6815744

            def pb_slice(base, half, nbytes):
                return pair_g[half * PB + base:half * PB + base + nbytes]

            def pb_f32(base, half, nbytes, rows):
                ap = pb_slice(base, half, nbytes).bitcast(f32)
                return ap.rearrange("(p n) -> p n", p=rows)

            def pb_u8(base, half, nbytes, rows):
                return pb_slice(base, half, nbytes).rearrange(
                    "(p n) -> p n", p=rows)

            # ordering matters: earlier phases' data gathers first
            for (i_t, g_t, grp) in (
                    (wqq_i, wq_g, halves), (wkq_i, wk_g, halves),
                    (pair_i, pair_g, pairs),
                    (wvq_i, wv_g, halves)):
                nc.gpsimd.collective_compute(
                    "AllGather", mybir.AluOpType.bypass, replica_groups=grp,
                    ins=[i_t[:].opt()], outs=[g_t[:].opt()])
            # fc quarter is uploaded as [heads0-2 part | heads3-5 part] so two
            # sliced gathers land head-aligned; W for heads 0-2 unblocks early
            HQ = QW // 2  # 576
            for piece in range(2):
                nc.gpsimd.collective_compute(
                    "AllGather", mybir.AluOpType.bypass, replica_groups=halves,
                    ins=[fcq_i[piece * HQ:(piece + 1) * HQ, :].opt()],
                    outs=[fc_g[piece * 4 * HQ:(piece + 1) * 4 * HQ, :].opt()])

            with ExitStack() as ab:   # pools that live through phases A+B only
                abp = ab.enter_context(tc.tile_pool(name="abp", bufs=1))

                # ---------- constants ----------
                ones_t = abp.tile([1, 128], f32, tag="ones")
                nc.vector.memset(ones_t[:], 1.0)
                km_t = abp.tile([1, LK], f32, tag="km")
                nc.gpsimd.dma_start(km_t[:], km_ext[:])

                kb_t = abp.tile([128, LK], f32, tag="kb")
                with tc.tile_pool(name="kbp", bufs=1, space="PSUM") as kbp:
                    kb_ps = kbp.tile([128, LK], f32, tag="kb_ps")
                    for c in range(2):
                        nc.tensor.matmul(kb_ps[:, c * 512:(c + 1) * 512], ones_t[:],
                                         km_t[:, c * 512:(c + 1) * 512],
                                         start=True, stop=True)
                    nc.scalar.copy(kb_t[:], kb_ps[:])

                # ---------- phase A: qhT/khT projections (fp32) ----------
                qhT_t = abp.tile([128, MT, LQ], f32, tag="qhT")
                khT_t = abp.tile([128, MT, LK], f32, tag="khT")
                with tc.tile_pool(name="pa_sb", bufs=1) as pa_sb, \
                     tc.tile_pool(name="pa_ps", bufs=2, space="PSUM") as pa_ps:
                    wq_t = pa_sb.tile([128, ET, HPC * DK], f32, tag="wq")
                    wk_t = pa_sb.tile([128, ET, HPC * DK], f32, tag="wk")
                    qTs = pa_sb.tile([128, ET, LQ], f32, tag="qTs")
                    kTs = pa_sb.tile([128, ET, LK], f32, tag="kTs")
                    for kt in range(ET):
                        for qu in range(4):
                            nc.gpsimd.dma_start(
                                wq_t[:, kt, qu * QWK:(qu + 1) * QWK],
                                wq_g[E * qu + kt * 128:E * qu + (kt + 1) * 128, :])
                            nc.gpsimd.dma_start(
                                wk_t[:, kt, qu * QWK:(qu + 1) * QWK],
                                wk_g[E * qu + kt * 128:E * qu + (kt + 1) * 128, :])
                        nc.gpsimd.dma_start(
                            qTs[:, kt, :],
                            pb_f32(O_QT + (kt % 3) * 524288, kt // 3, 524288, 128))
                        nc.gpsimd.dma_start(
                            kTs[:, kt, :],
                            pb_f32(O_KT + (kt % 3) * 524288, kt // 3, 524288, 128))
                    for (w_t, src, dst) in ((wq_t, qTs, qhT_t), (wk_t, kTs, khT_t)):
                        for mt in range(MT):
                            for c in range(2):
                                ps = pa_ps.tile([128, 512], f32, tag="pa")
                                for kt in range(ET):
                                    nc.tensor.matmul(
                                        ps[:],
                                        w_t[:, kt, mt * 128:(mt + 1) * 128],
                                        src[:, kt, c * 512:(c + 1) * 512],
                                        start=(kt == 0), stop=(kt == ET - 1))
                                nc.scalar.copy(dst[:, mt, c * 512:(c + 1) * 512],
                                               ps[:])

                # ---------- phase B: scores + argmax (fp32) ----------
                with tc.tile_pool(name="pb_sb", bufs=2) as pb_sb, \
                     tc.tile_pool(name="pb_s", bufs=3) as pb_s, \
                     tc.tile_pool(name="pb_ps", bufs=4, space="PSUM") as pb_ps:
                    for qt in range(QT):
                        qk_t = pb_sb.tile([128, LK], f32, tag="qk")
                        post_t = pb_sb.tile([128, LK], f32, tag="post")
                        nc.gpsimd.dma_start(
                            qk_t[:],
                            pb_f32(O_QK + (qt % 4) * 524288, qt // 4, 524288, 128))
                        nc.gpsimd.dma_start(
                            post_t[:],
                            pb_u8(O_PO + (qt % 4) * 131072, qt // 4, 131072, 128))
                        pen_t = pb_sb.tile([128, LK], f32, tag="pen")
                        nc.scalar.activation(pen_t[:], post_t[:],
                                             mybir.ActivationFunctionType.Copy,
                                             bias=-1e9, scale=1e9)
                        comb_t = pb_sb.tile([128, LK], f32, tag="comb")
                        nc.vector.tensor_add(comb_t[:], qk_t[:], pen_t[:])
                        nc.vector.tensor_add(comb_t[:], comb_t[:], kb_t[:])

                        for hp in range(MT):  # head pairs, packed on row groups
                            accs = [pb_ps.tile([128, LK], f32, tag="acc",
                                               name=f"acc{i}")
                                    for i in range(2)]
                            for c in range(2):
                                for hi in range(2):
                                    nc.tensor.matmul(
                                        accs[hi][:, c * 512:(c + 1) * 512],
                                        qhT_t[64 * hi:64 * (hi + 1), hp,
                                              qt * 128:(qt + 1) * 128],
                                        khT_t[64 * hi:64 * (hi + 1), hp,
                                              c * 512:(c + 1) * 512],
                                        start=True, stop=True)
                            for hi in range(2):
                                h = 2 * hp + hi
                                s_t = pb_s.tile([128, LK], f32, tag="s")
                                nc.vector.tensor_add(s_t[:], accs[hi][:], comb_t[:])
                                max8 = pb_s.tile([128, 8], f32, tag="max8")
                                idx8 = pb_s.tile([128, 8], u16, tag="idx8")
                                nc.vector.max(max8[:], s_t[:])
                                nc.vector.max_index(idx8[:], max8[:], s_t[:])
                                nc.vector.tensor_copy(jtiles[:, h, qt:qt + 1],
                                                      idx8[:, 0:1])

            # j* to DRAM contiguously; read back in the gather ucode's index
            # layout with a fixed slot permutation (undone on the host):
            # slot i (partition i%16, col i//16) of head h holds
            # j*[h, q = ((i//16)%8)*128 + (i//128)*16 + (i%16)].
            jscr = jdram.tile([128, HPC * QT], u16, tag="jscr")
            nc.gpsimd.dma_start(jout_ext[:], jtiles[:])
            nc.gpsimd.dma_start(jscr[:],
                                jtiles[:].rearrange("p h qt -> p (h qt)"))
            for g in range(8):
                nc.gpsimd.dma_start(
                    idx_all[16 * g:16 * (g + 1), :].rearrange(
                        "p (h clo chi) -> p h clo chi", h=HPC, clo=QT),
                    jscr[:].rearrange("(clo p) (h chi) -> p h clo chi",
                                      clo=QT, h=HPC))

            # ---------- phase C: value path (float32r) + gather ----------
            vT_r = const.tile([128, DVT, LK], f32r, tag="vT_r")
            for kt in range(DVT):
                nc.gpsimd.dma_start(
                    vT_r[:, kt, :],
                    pb_f32(O_VT + (kt % 3) * 524288, kt // 3, 524288, 128))

            part_t = const.tile([128, QT * DV], f32, tag="part")
            with tc.tile_pool(name="pc_w", bufs=2) as pc_w, \
                 tc.tile_pool(name="pc_vh", bufs=1) as pc_vh, \
                 tc.tile_pool(name="pc_ev", bufs=2) as pc_ev, \
                 tc.tile_pool(name="pc_ps", bufs=2, space="PSUM") as pc_ps, \
                 tc.tile_pool(name="pc_g", bufs=1) as pc_g:
                for h in range(HPC):
                    wv_t = pc_w.tile([128, DVT, DV], f32r, tag="wv")
                    fc_t = pc_w.tile([128, DVT, DV], f32r, tag="fc")
                    for kt in range(DVT):
                        # wv_g rows [768*qu + dv], cols local to quarter qu
                        done = 0
                        while done < DV:
                            gcol = h * DV + done
                            qu, off = divmod(gcol, QW)
                            seg = min(QW - off, DV - done)
                            nc.gpsimd.dma_start(
                                wv_t[:, kt, done:done + seg],
                                wv_g[DV * qu + kt * 128:DV * qu + (kt + 1) * 128,
                                     off:off + seg])
                            done += seg
                        nc.gpsimd.dma_start(
                            fc_t[:, kt, :],
                            fc_g[h * DV + kt * 128:h * DV + (kt + 1) * 128, :])

                    # vh^T = wv_h.T @ vT  -> [hd, j]
                    vhT_t = pc_vh.tile([128, DVT, LK], f32r, tag="vhT")
                    for mt in range(DVT):
                        for c in range(2):
                            ps = pc_ps.tile([128, 512], f32, tag="vh_ps")
                            for kt in range(DVT):
                                nc.tensor.matmul(
                                    ps[:],
                                    wv_t[:, kt, mt * 128:(mt + 1) * 128],
                                    vT_r[:, kt, c * 512:(c + 1) * 512],
                                    start=(kt == 0), stop=(kt == DVT - 1))
                            nc.scalar.copy(vhT_t[:, mt, c * 512:(c + 1) * 512], ps[:])

                    # W_h = vh^T.T @ fc_h -> [j, o], row-major to DRAM
                    wbuf = dram.tile([LK, DV], f32, tag="wbuf")
                    for jt in range(QT):
                        ps = pc_ps.tile([128, DV], f32, tag="w_ps")
                        for (o0, o1) in ((0, 512), (512, DV)):
                            for kt in range(DVT):
                                nc.tensor.matmul(
                                    ps[:, o0:o1],
                                    vhT_t[:, kt, jt * 128:(jt + 1) * 128],
                                    fc_t[:, kt, o0:o1],
                                    start=(kt == 0), stop=(kt == DVT - 1))
                        wev = pc_ev.tile([128, DV], f32, tag="wev")
                        nc.scalar.copy(wev[:], ps[:])
                        nc.gpsimd.dma_start(wbuf[jt * 128:(jt + 1) * 128, :], wev[:])

                    # gather W rows at j* and accumulate
                    gout = pc_g.tile([128, QT, DV], f32, tag="gout")
                    nc.gpsimd.dma_gather(
                        gout[:], wbuf[:],
                        idx_all[:, h * (LQ // 16):(h + 1) * (LQ // 16)],
                        num_idxs=LQ, num_idxs_reg=LQ, elem_size=DV)
                    gflat = gout[:].rearrange("p qt o -> p (qt o)")
                    if h == 0:
                        nc.vector.tensor_copy(part_t[:], gflat)
                    else:
                        nc.vector.tensor_add(part_t[:], part_t[:], gflat)

            nc.gpsimd.dma_start(part_ext[:],
                                part_t[:].rearrange("p (qt o) -> p qt o", qt=QT))

    nc.compile()
    return nc


def _make_in_maps(q, k, v, qk_mask, k_mask, post, w_qs, w_ks, w_vs, fc):
    wq8 = w_qs / np.float32(8.0)   # fold 1/sqrt(DK); exact power-of-2 scale
    qT = [np.ascontiguousarray(q[b].T) for b in range(B)]
    kT = [np.ascontiguousarray(k[b].T) for b in range(B)]
    vT = [np.ascontiguousarray(v[b].T) for b in range(B)]
    post_u8 = post.astype(np.uint8)
    in_maps = []
    for c in range(N_CORES):
        b, hh = c // 2, c % 2
        QW = H * DV // 8
        QWK = HPC * DK // 4
        qs = slice(hh * HPC * DV + b * QW, hh * HPC * DV + (b + 1) * QW)
        qsk = slice(hh * HPC * DK + b * QWK, hh * HPC * DK + (b + 1) * QWK)
        eh = slice(hh * E // 2, (hh + 1) * E // 2)
        qh2 = slice(hh * LQ // 2, (hh + 1) * LQ // 2)
        pairb = np.concatenate([
            np.ascontiguousarray(qT[b][eh]).view(np.uint8).ravel(),
            np.ascontiguousarray(kT[b][eh]).view(np.uint8).ravel(),
            np.ascontiguousarray(vT[b][eh]).view(np.uint8).ravel(),
            np.ascontiguousarray(qk_mask[b, 0, qh2]).view(np.uint8).ravel(),
            np.ascontiguousarray(post_u8[b, 0, qh2]).ravel(),
        ])
        in_maps.append({
            "pairb": pairb,
            "wq": np.ascontiguousarray(wq8[:, qsk]),
            "wk": np.ascontiguousarray(w_ks[:, qsk]),
            "wv": np.ascontiguousarray(w_vs[:, qs]).astype(np.float16),
            "fc": np.concatenate([
                fc[hh * HPC * DV + b * QW // 2:
                   hh * HPC * DV + (b + 1) * QW // 2, :],
                fc[hh * HPC * DV + HPC * DV // 2 + b * QW // 2:
                   hh * HPC * DV + HPC * DV // 2 + (b + 1) * QW // 2, :],
            ]).astype(np.float16),
            "km": np.ascontiguousarray(k_mask[b, 0, :, 0][None, :]),
        })
    return in_maps


def _get_runner():
    """Compiled program + jitted shard_map executor + cached device zeros."""
    if "runner" in _compiled:
        return _compiled["runner"]
    import jax
    from jax.sharding import Mesh, PartitionSpec
    from jax.experimental.shard_map import shard_map
    import concourse.mybir as mybir
    from concourse.bass2jax import (_bass_exec_p, partition_id_tensor,
                                    install_neuronx_cc_hook)

    nc = _build_program()
    install_neuronx_cc_hook()
    partition_name = nc.partition_id_tensor.name if nc.partition_id_tensor else None
    in_names, out_names, out_avals = [], [], []
    for alloc in nc.m.functions[0].allocations:
        if not isinstance(alloc, mybir.MemoryLocationSet):
            continue
        name = alloc.memorylocations[0].name
        if alloc.kind == "ExternalInput":
            if name != partition_name:
                in_names.append(name)
        elif alloc.kind == "ExternalOutput":
            out_names.append(name)
            out_avals.append(jax.core.ShapedArray(tuple(alloc.tensor_shape),
                                                  mybir.dt.np(alloc.dtype)))
    all_in_names = list(in_names) + list(out_names)
    if partition_name is not None:
        all_in_names.append(partition_name)

    def _body(*args):
        operands = list(args)
        if partition_name is not None:
            operands.append(partition_id_tensor())
        outs = _bass_exec_p.bind(
            *operands,
            out_avals=tuple(out_avals),
            in_names=tuple(all_in_names),
            out_names=tuple(out_names),
            lowering_input_output_aliases=(),
            sim_require_finite=True,
            sim_require_nnan=True,
            nc=nc,
        )
        return tuple(outs)

    devices = jax.devices()[:N_CORES]
    mesh = Mesh(np.asarray(devices), ("core",))
    n_outs = len(out_names)
    in_specs = (PartitionSpec("core"),) * (len(in_names) + n_outs)
    out_specs = (PartitionSpec("core"),) * n_outs
    fn = jax.jit(shard_map(_body, mesh=mesh, in_specs=in_specs,
                           out_specs=out_specs, check_rep=False),
                 keep_unused=True)
    # device-resident zero output images (not donated -> reusable)
    sharding = jax.sharding.NamedSharding(mesh, PartitionSpec("core"))
    zeros = [jax.device_put(
                np.zeros((N_CORES * a.shape[0], *a.shape[1:]), a.dtype), sharding)
             for a in out_avals]
    _compiled["runner"] = (fn, in_names, out_names, out_avals, mesh, devices,
                           sharding, zeros)
    return _compiled["runner"]


def kernel(q, k, v, qpos, kpos, qk_mask, k_mask, post_softmax_mask,
           w_qs, w_ks, w_vs, fc):
    import jax
    from jax.sharding import PartitionSpec

    fn, in_names, out_names, out_avals, mesh, devices, sharding, zeros = \
        _get_runner()

    q = np.asarray(q, np.float32)
    k = np.asarray(k, np.float32)
    v = np.asarray(v, np.float32)
    qk_mask = np.asarray(qk_mask, np.float32)
    k_mask = np.asarray(k_mask, np.float32)
    post = np.asarray(post_softmax_mask, np.float32)
    w_qs = np.asarray(w_qs, np.float32)
    w_ks = np.asarray(w_ks, np.float32)
    w_vs = np.asarray(w_vs, np.float32)
    fc = np.asarray(fc, np.float32)

    in_maps = _make_in_maps(q, k, v, qk_mask, k_mask, post, w_qs, w_ks, w_vs, fc)

    # upload each shard asynchronously, then assemble global arrays
    glob = []
    for nm in in_names:
        shards = [jax.device_put(in_maps[c][nm], devices[c])
                  for c in range(N_CORES)]
        s0 = in_maps[0][nm].shape
        glob.append(jax.make_array_from_single_device_arrays(
            (N_CORES * s0[0], *s0[1:]), sharding, shards))

    outs = fn(*glob, *zeros)
    res = {nm: np.asarray(o).reshape(N_CORES, *out_avals[i].shape)
           for i, (nm, o) in enumerate(zip(out_names, outs))}

    output = np.empty((B, LQ, DV), np.float32)
    attn = np.zeros((B, H, LQ, LK), np.float32)
    qidx = np.arange(LQ)
    ii = np.arange(LQ)
    qmap = ((ii // 16) % QT) * 128 + (ii // 128) * 16 + (ii % 16)
    for b in range(B):
        out_b = np.zeros((LQ, DV), np.float32)
        for hh in (0, 1):
            pf = res["part"][2 * b + hh].transpose(1, 0, 2).reshape(LQ, DV)
            out_b[qmap] += pf
            jt = res["jout"][2 * b + hh]  # [128, HPC, QT], q = qt*128 + p
            for h in range(HPC):
                j = jt[:, h, :].T.reshape(LQ).astype(np.int64)
                attn[b, hh * HPC + h, qidx, j] = 1.0
        output[b] = out_b + v[b]
    return output, attn
